# revision 25
# baseline (speedup 1.0000x reference)
"""Trainium2 Bass kernel for batched per-frame LPC synthesis + windowed overlap-add.

v2 — restructured from the v1 FFT kernel for engine balance:
  * The forward FFT of each 128-sample block is computed ONCE per block and
    shared by the 4 overlapping frames that contain it (v1 recomputed it 4x).
  * Spectral multiply Y = X*H runs on DVE in bf16 (2x packed mode) with
    row-fused long instructions; a "shifted H" copy keeps every operand
    4B-aligned for the packed mode regardless of the chunk offset parity.
  * The inverse DFT matmuls accumulate DIRECTLY into final output blocks in
    PSUM (overlap-add across frames is re-associated into the matmul
    accumulation), eliminating v1's per-chunk PSUM->SBUF copies and the
    separate shifted-add stage.  The Hann window stays folded into the 32
    inverse matrices; 1/norm is applied by the PSUM-draining DVE multiply.
  * Elementwise work is split DVE/Pool/Act by measured engine rates.

  Data parallel over the batch: 16 rows -> 8 cores x 2 rows.
"""

import numpy as np
import ml_dtypes

import concourse.bass as bass
import concourse.tile as tile
from concourse import bacc
from concourse import mybir
from concourse.bass_utils import run_bass_kernel_spmd
from concourse.masks import make_identity

# problem constants (hardcoded per contract)
HOP, WIN, PAD = 256, 1024, 384
B, T, P = 16, 262144, 22
F = T // HOP              # 1024 frames per row
NFFT = 256
TB = T // 128             # 2048 raw 128-blocks per row
NCORES = 8
BPC = B // NCORES         # 2 batch rows per core
FC = BPC * F              # 2048 frames per core
FTS = 512                 # frames per tile (one PSUM bank at fp32)
XTW = 2056                # xt width: TB + 3 left margin + 5 right
MG = 4                    # X/H/Y tile margins (cols per side)
HW_ = F + 2 * MG          # 1032: per-row width of X/H/Y tiles
J0, J1 = 2, 2 * HW_ - 2   # spectral op col range (both-rows fused)

_f32 = mybir.dt.float32
_f32r = mybir.dt.float32r
_bf16 = mybir.dt.bfloat16

# chunk geometry: chunk c of frame f is raw block b = 2f + c - 3
_PAR = [(c + 1) % 2 for c in range(8)]            # block parity per chunk
_S = [(c - 3 - _PAR[c]) // 2 for c in range(8)]   # X col shift per chunk
_SHIFTED = [s % 2 != 0 for s in _S]               # odd shift -> shifted Y/H


def _r(ap):
    return ap.bitcast(_f32r) if ap.dtype == _f32 else ap


# ---------------------------------------------------------------- constants
def _build_consts():
    n_ = np.arange(128)
    k_ = np.arange(128)
    win = 0.5 * (1.0 - np.cos(2.0 * np.pi * np.arange(WIN) / WIN))

    ang = 2 * np.pi * np.outer(n_, k_) / NFFT
    Fr = np.cos(ang)
    Fi = -np.sin(ang)
    Fi[:, 0] = (-1.0) ** n_                      # slot0: X[128] into Xi[0]

    m_ = np.arange(1, P + 1)
    angA = 2 * np.pi * np.outer(m_, k_) / NFFT
    Ar = np.vstack([np.ones(128), np.cos(angA)])     # [23, 128]
    Ai = np.vstack([np.zeros(128), -np.sin(angA)])
    Ai[:, 0] = (-1.0) ** np.arange(0, P + 1)         # col0: A[128]

    nn = np.arange(256)
    angI = 2 * np.pi * np.outer(k_, nn) / NFFT
    Cr = 2 * np.cos(angI) / NFFT
    Ci = -2 * np.sin(angI) / NFFT
    Cr[0, :] = 1.0 / NFFT
    Ci[0, :] = ((-1.0) ** nn) / NFFT
    INV = np.zeros((128, 8, 4, 128), np.float64)  # [k, c, v, n]
    for blk in range(8):
        wseg = win[128 * blk: 128 * (blk + 1)]
        INV[:, blk, 0, :] = Cr[:, :128] * wseg       # lo-r (chunk = c)
        INV[:, blk, 1, :] = Ci[:, :128] * wseg       # lo-i
        INV[:, blk, 2, :] = Cr[:, 128:] * wseg       # hi-r (chunk = c-1)
        INV[:, blk, 3, :] = Ci[:, 128:] * wseg       # hi-i

    # norm reciprocal, parity split: nrp[p][n, u] = 1/norm at block 2u+p
    idx = (np.arange(F)[:, None] * HOP + np.arange(WIN)[None, :]).reshape(-1)
    L = (F - 1) * HOP + WIN
    norm = np.zeros(L)
    np.add.at(norm, idx, np.tile(win, F))
    nr = (1.0 / norm[PAD:PAD + T]).reshape(TB, 128).T   # [n, block]

    bf = ml_dtypes.bfloat16
    f32 = np.float32
    return {
        "frb": np.ascontiguousarray(Fr, bf),
        "fib": np.ascontiguousarray(Fi, bf),
        "arw": np.ascontiguousarray(Ar, f32),
        "aiw": np.ascontiguousarray(Ai, f32),
        "invw": np.ascontiguousarray(INV.reshape(128, 32 * 128), bf),
        "nre": np.ascontiguousarray(nr[:, 0::2], bf),
        "nro": np.ascontiguousarray(nr[:, 1::2], bf),
    }


# ---------------------------------------------------------------- program
def _emit(nc):
    ex_d = nc.dram_tensor("ex2", [BPC, T], _f32, kind="ExternalInput")
    at_d = nc.dram_tensor("atc", [P + 1, FC], _f32, kind="ExternalInput")
    fr_d = nc.dram_tensor("frb", [128, 128], _bf16, kind="ExternalInput")
    fi_d = nc.dram_tensor("fib", [128, 128], _bf16, kind="ExternalInput")
    ar_d = nc.dram_tensor("arw", [P + 1, 128], _f32, kind="ExternalInput")
    ai_d = nc.dram_tensor("aiw", [P + 1, 128], _f32, kind="ExternalInput")
    inv_d = nc.dram_tensor("invw", [128, 32 * 128], _bf16, kind="ExternalInput")
    nre_d = nc.dram_tensor("nre", [128, TB // 2], _bf16, kind="ExternalInput")
    nro_d = nc.dram_tensor("nro", [128, TB // 2], _bf16, kind="ExternalInput")
    h0_d = nc.dram_tensor("h0", [1, 2 * BPC * HW_], _f32, kind="ExternalInput")
    out_d = nc.dram_tensor("out", [BPC, T], _f32, kind="ExternalOutput")

    with tile.TileContext(nc) as tc:
        _body(nc, tc, ex_d, at_d, fr_d, fi_d, ar_d, ai_d, inv_d,
              nre_d, nro_d, h0_d, out_d)
    return nc


def _drain_row(nc, r, ob, accs, nre, nro, ost, ps, identf, out_d):
    """1/norm multiply (PSUM drain) + de-transpose + 8-way split store."""
    obv = ob[r].rearrange("p (a two) -> p a two", two=2)
    for par in range(2):
        nrt = nre if par == 0 else nro
        for ft in range(2):
            nc.vector.tensor_mul(
                obv[:, bass.ts(ft, FTS), par],
                accs[(r, par, ft)], nrt[:, bass.ts(ft, FTS)])
    for g in range(4):
        st = ost.tile([128, 4, 128], _f32, tag="st")
        pt = ps("ptr")
        for mq in range(4):
            m = 4 * g + mq
            nc.tensor.transpose(
                pt[:, bass.ts(mq, 128)], ob[r][:, bass.ts(m, 128)], identf)
        nc.scalar.copy(st.rearrange("p a b -> p (a b)"), pt)
        for h in range(2):
            nc.sync.dma_start(
                out_d.ap()[r, bass.ds(65536 * g + 32768 * h, 32768)].rearrange(
                    "(m tl n) -> tl m n", m=2, tl=128),
                st[:, bass.ts(h, 2)],
            )


def _body(nc, tc, ex_d, at_d, fr_d, fi_d, ar_d, ai_d, inv_d,
          nre_d, nro_d, h0_d, out_d):
    from contextlib import ExitStack

    with ExitStack() as ctx:
        consts = ctx.enter_context(tc.tile_pool(name="consts", bufs=1))
        big = ctx.enter_context(tc.tile_pool(name="big", bufs=1))
        atp = ctx.enter_context(tc.tile_pool(name="atp", bufs=2))
        xtp = ctx.enter_context(tc.tile_pool(name="xtp", bufs=2))
        raw = ctx.enter_context(tc.tile_pool(name="raw", bufs=16))
        hwk = ctx.enter_context(tc.tile_pool(name="hwk", bufs=1))
        twk = ctx.enter_context(tc.tile_pool(name="twk", bufs=2))
        obp = ctx.enter_context(tc.tile_pool(name="obp", bufs=2))
        ost = ctx.enter_context(tc.tile_pool(name="ost", bufs=2))
        psp = ctx.enter_context(tc.tile_pool(name="psp", bufs=8, space="PSUM"))

        def ps(name):
            return psp.tile([128, FTS], _f32, tag="ps", name=name)

        # ---- constants into SBUF ----
        frb = consts.tile([128, 128], _bf16, tag="frb")
        fib = consts.tile([128, 128], _bf16, tag="fib")
        arw = consts.tile([P + 1, 128], _f32r, tag="arw")
        aiw = consts.tile([P + 1, 128], _f32r, tag="aiw")
        invw = consts.tile([128, 32, 128], _bf16, tag="invw")
        nre = consts.tile([128, TB // 2], _bf16, tag="nre")
        nro = consts.tile([128, TB // 2], _bf16, tag="nro")
        h0t = consts.tile([1, 2, BPC, HW_], _f32, tag="h0t")
        identf = consts.tile([128, 128], _f32, tag="identf")
        nc.sync.dma_start(frb, fr_d.ap())
        nc.sync.dma_start(fib, fi_d.ap())
        nc.sync.dma_start(arw, ar_d.ap().bitcast(_f32r))
        nc.sync.dma_start(aiw, ai_d.ap().bitcast(_f32r))
        for iw in range(4):
            nc.sync.dma_start(
                invw[:, bass.ts(iw, 8)],
                inv_d.ap()[:, bass.ts(iw, 8 * 128)].rearrange(
                    "k (i n) -> k i n", n=128))
        nc.sync.dma_start(nre, nre_d.ap())
        nc.sync.dma_start(nro, nro_d.ap())
        nc.sync.dma_start(h0t, h0_d.ap().rearrange("q (s a b) -> q s a b", s=2, b=HW_))
        make_identity(nc, identf)

        # ---- persistent per-core tensors ----
        xre = big.tile([128, BPC, HW_], _bf16, tag="xre")
        xro = big.tile([128, BPC, HW_], _bf16, tag="xro")
        xie = big.tile([128, BPC, HW_], _bf16, tag="xie")
        xio = big.tile([128, BPC, HW_], _bf16, tag="xio")
        hra = big.tile([128, BPC, HW_], _bf16, tag="hra")
        hrb = big.tile([128, BPC, HW_], _bf16, tag="hrb")
        his = big.tile([128, BPC, HW_], _bf16, tag="his")
        hras = big.tile([128, BPC, HW_], _bf16, tag="hras")
        hrbs = big.tile([128, BPC, HW_], _bf16, tag="hrbs")
        hiss = big.tile([128, BPC, HW_], _bf16, tag="hiss")
        yr = [big.tile([128, BPC, HW_], _bf16, tag=f"yr{c}", name=f"yr{c}")
              for c in range(8)]
        yi = [big.tile([128, BPC, HW_], _bf16, tag=f"yi{c}", name=f"yi{c}")
              for c in range(8)]
        ob = [obp.tile([128, TB], _f32, tag="ob", name="ob0"),
              obp.tile([128, TB], _f32, tag="ob", name="ob1")]


        # ---- input DMAs for both rows, issued up-front ----
        rts = [[None] * 8 for _ in range(BPC)]
        for r in range(BPC):
            for s8 in range(8):
                rt = raw.tile([128, 2, 128], _f32, tag="rt")
                rts[r][s8] = rt
                eng = nc.scalar if (s8 % 2 == 0) else nc.sync
                eng.dma_start(
                    rt,
                    ex_d.ap()[r, bass.ts(s8, 32768)].rearrange(
                        "(u p j) -> p u j", u=2, p=128),
                )

        # ---- H stage: per-frame filter spectrum H = g / A(w^k) ----
        for ft in range(FC // FTS):
            rsel, fo = ft // (F // FTS), (ft % (F // FTS)) * FTS
            atc = atp.tile([P + 1, FTS], _f32r, tag="atc")
            nc.sync.dma_start(atc, at_d.ap()[:, bass.ts(ft, FTS)].bitcast(_f32r))
            pbr = ps("pbr")
            pbi = ps("pbi")
            nc.tensor.matmul(pbr, arw, atc, start=True, stop=True)
            nc.tensor.matmul(pbi, aiw, atc, start=True, stop=True)
            brs = hwk.tile([128, FTS], _f32, tag="brs")
            bis = hwk.tile([128, FTS], _f32, tag="bis")
            nc.scalar.copy(brs, pbr)
            nc.scalar.copy(bis, pbi)
            t3 = hwk.tile([128, FTS], _f32, tag="t3")
            t4 = hwk.tile([128, FTS], _f32, tag="t4")
            nc.vector.tensor_mul(t3, brs, brs)
            nc.vector.tensor_mul(t4, bis, bis)
            nc.vector.tensor_add(t3, t3, t4)
            nc.vector.reciprocal_approx_fast(t4, t3)
            ds = bass.ds(MG + fo, FTS)
            nc.vector.tensor_mul(hra[:, rsel, ds], brs, t4)
            nc.vector.tensor_mul(his[:, rsel, ds], bis, t4)
        # margins: zero once; interior writes never touch them
        for tl in [xre, xro, xie, xio, hra, hrb, his]:
            fl = tl.rearrange("p a b -> p (a b)")
            nc.gpsimd.memset(fl[:, 0:MG], 0.0)
            nc.gpsimd.memset(fl[:, HW_ - MG:HW_ + MG], 0.0)
            nc.gpsimd.memset(fl[:, 2 * HW_ - MG:2 * HW_], 0.0)
        for tl in yr + yi:
            fl = tl.rearrange("p a b -> p (a b)")
            # cols {0,1, 1030,1031, 1032,1033, 2062,2063}
            v = fl.rearrange("p (a b) -> p a b", b=2)
            nc.gpsimd.memset(v[:, 0:1, :], 0.0)
            nc.gpsimd.memset(v[:, 515:517, :], 0.0)
            nc.gpsimd.memset(v[:, 1031:1032, :], 0.0)
        # slot-0 rows: his[0]=0; hra[0]=H(1), hrb[0]=H(-1) from host; the
        # hrb body rows are a copy of hra (both are Re(H) off slot 0)
        nc.gpsimd.memset(his[0:1], 0.0)
        nc.scalar.copy(hra[0:1], h0t[:, 0])

        def _h_shifts():
            # hrb + shifted-H copies, emitted after row-0 X so the Act engine
            # prioritizes the input pipeline
            nc.scalar.copy(hrb, hra)
            nc.scalar.copy(hrb[0:1], h0t[:, 1])
            W2 = 2 * HW_
            for src_, dst in ((hra, hras), (hrb, hrbs), (his, hiss)):
                sf = src_.rearrange("p a b -> p (a b)")
                df = dst.rearrange("p a b -> p (a b)")
                nc.scalar.copy(df[:, 0:W2 - 1], sf[:, 1:W2])
                nc.gpsimd.memset(df[:, W2 - 1:W2], 0.0)

        # ---- transpose + shared forward FFT (per row) ----
        for r in range(BPC):
            xt = xtp.tile([128, XTW], _bf16, tag="xt")
            nc.vector.memset(xt[:, 0:3], 0.0)
            nc.vector.memset(xt[:, 3 + TB:XTW], 0.0)
            for s4 in range(4):
                pt = ps("pt")
                for h in range(2):
                    for q in range(2):
                        nc.tensor.transpose(
                            pt[:, bass.ds(256 * h + 128 * q, 128)],
                            rts[r][2 * s4 + h][:, q], identf)
                nc.scalar.copy(
                    xt[:, 3 + 512 * s4: 3 + 512 * (s4 + 1)], pt)
            xtv = xt.rearrange("p (t two) -> p two t", two=2)
            # parity-direct forward FFT: q=1 cols are even blocks b=2t'-2,
            # q=0 cols are odd blocks b=2t'-3
            tail = ps("tail")
            ti = 0
            for q, coff, xr_, xi_ in ((1, MG - 1, xre, xie),
                                      (0, MG - 2, xro, xio)):
                for w, xdst in ((frb, xr_), (fib, xi_)):
                    for k in range(2):
                        px = ps("px")
                        nc.tensor.matmul(px, w, xtv[:, q, bass.ts(k, 512)],
                                         start=True, stop=True)
                        nc.scalar.copy(
                            xdst[:, r, bass.ds(coff + 512 * k, 512)], px)
                    # 4-col tail piece into shared tail psum
                    nc.tensor.matmul(tail[:, bass.ds(4 * ti, 4)], w,
                                     xtv[:, q, bass.ds(1024, 4)],
                                     start=True, stop=True)
                    nc.scalar.copy(xdst[:, r, bass.ds(coff + 1024, 4)],
                                   tail[:, bass.ds(4 * ti, 4)])
                    ti += 1
            if r == 0:
                _h_shifts()

        # ---- spectral multiply + inverse DFT/overlap-add, chunk-pipelined --
        accs = {}
        nstop = {}
        for r in range(BPC):
            for par in range(2):
                for ft in range(2):
                    cs = [c for c in range(8) if c % 2 == (par + 3) % 2]
                    n = sum(2 if (c == 0) else 4 for c in cs)
                    nstop[(r, par, ft)] = n

        WSP = HW_ - MG  # 1028: spectral op width per row
        nterm = {}
        for r in range(BPC):
            for c in range(8):
                if r == 1 and c == 1:
                    # row-0 drain + store overlaps row-1 spectral/inverse
                    _drain_row(nc, 0, ob, accs, nre, nro, ost, ps, identf,
                               out_d)
                par, s, sh = _PAR[c], _S[c], _SHIFTED[c]
                xr_ = xre if par == 0 else xro
                xi_ = xie if par == 0 else xio
                h1 = hras if sh else hra
                h2 = hrbs if sh else hrb
                h3 = hiss if sh else his
                so = s + (1 if sh else 0)
                xsl = bass.ds(J0 + so, WSP)
                hsl = bass.ds(J0, WSP)
                t1 = twk.tile([128, WSP], _bf16, tag="t1")
                t2 = twk.tile([128, WSP], _bf16, tag="t2")
                t3_ = twk.tile([128, WSP], _bf16, tag="t3")
                t4_ = twk.tile([128, WSP], _bf16, tag="t4")
                nc.vector.tensor_mul(t1, xr_[:, r, xsl], h1[:, r, hsl])
                nc.vector.tensor_mul(t2, xi_[:, r, xsl], h3[:, r, hsl])
                nc.vector.tensor_mul(t3_, xi_[:, r, xsl], h2[:, r, hsl])
                nc.vector.tensor_mul(t4_, xr_[:, r, xsl], h3[:, r, hsl])
                nc.vector.tensor_add(yr[c][:, r, hsl], t1, t2)
                nc.vector.tensor_sub(yi[c][:, r, hsl], t3_, t4_)

                # inverse terms that become ready with this chunk:
                # lo terms of chunk c and hi terms of chunk c (reading c-1)
                cpar = (c + 1) % 2   # block parity this chunk contributes to
                for ft in range(2):
                    key = (r, cpar, ft)
                    if key not in accs:
                        accs[key] = ps(f"acc{r}{cpar}{ft}")
                        nterm[key] = 0
                    b0 = cpar + 1024 * ft
                    sc = (b0 + 3 - c) // 2
                    for v, cc in ((0, c), (1, c), (2, c - 1), (3, c - 1)):
                        if cc < 0:
                            continue
                        ysrc = yr[cc] if v % 2 == 0 else yi[cc]
                        col = MG + sc - (1 if _SHIFTED[cc] else 0)
                        nterm[key] += 1
                        nc.tensor.matmul(
                            accs[key], invw[:, 4 * c + v],
                            ysrc[:, r, bass.ds(col, FTS)],
                            start=(nterm[key] == 1),
                            stop=(nterm[key] == nstop[key]),
                        )
        _drain_row(nc, 1, ob, accs, nre, nro, ost, ps, identf, out_d)


# ---------------------------------------------------------------- entry
_prog = None


def _get_program():
    global _prog
    if _prog is None:
        nc = bacc.Bacc("TRN2", target_bir_lowering=False, debug=False)
        _prog = _emit(nc)
        nc.compile()
    return _prog


def build_in_maps(ex, gain, a):
    ex = np.ascontiguousarray(ex, np.float32)
    gain = np.ascontiguousarray(gain, np.float32)
    a = np.ascontiguousarray(a, np.float32)
    consts = _build_consts()

    # host prep of the tiny per-frame coefficient tensor: [1, a]/g -> [23, F]
    at = np.concatenate([np.ones((B, F, 1), np.float32), a], axis=2)
    at /= gain[:, :, None]
    # slot-0 filter rows, exact on host: H(w=1) = 1/sum(at), H(w=-1) alt-sum
    sgn = (-1.0) ** np.arange(P + 1, dtype=np.float32)
    br0 = at.sum(axis=2)
    bi0 = (at * sgn).sum(axis=2)
    h0 = np.zeros((B, 2, HW_), np.float32)
    h0[:, 0, MG:MG + F] = 1.0 / br0
    h0[:, 1, MG:MG + F] = 1.0 / bi0

    in_maps = []
    for c in range(NCORES):
        rows = slice(BPC * c, BPC * (c + 1))
        in_maps.append({
            "ex2": ex[rows],
            "atc": np.ascontiguousarray(
                at[rows].reshape(FC, P + 1).T, np.float32),
            "h0": np.ascontiguousarray(
                h0[rows].transpose(1, 0, 2).reshape(1, 2 * BPC * HW_)),
            **consts,
        })
    return in_maps


def kernel(ex: np.ndarray, gain: np.ndarray, a: np.ndarray) -> np.ndarray:
    nc = _get_program()
    in_maps = build_in_maps(ex, gain, a)
    res = run_bass_kernel_spmd(nc, in_maps, list(range(NCORES)))
    out = np.concatenate([res.results[i]["out"] for i in range(NCORES)], axis=0)
    return np.ascontiguousarray(out, np.float32)


if __name__ == "__main__":
    rng = np.random.default_rng(0)
    y = kernel(
        rng.standard_normal((B, T), dtype=np.float32),
        rng.uniform(0.1, 1.0, (B, F)).astype(np.float32),
        (rng.standard_normal((B, F, P), dtype=np.float32) * 0.01),
    )
    print(y.shape, y.dtype, float(np.abs(y).max()))


# revision 26
# speedup vs baseline: 1.0284x; 1.0284x over previous
"""Trainium2 Bass kernel for batched per-frame LPC synthesis + windowed overlap-add.

v2 — restructured from the v1 FFT kernel for engine balance:
  * The forward FFT of each 128-sample block is computed ONCE per block and
    shared by the 4 overlapping frames that contain it (v1 recomputed it 4x).
  * Spectral multiply Y = X*H runs on DVE in bf16 (2x packed mode) with
    row-fused long instructions; a "shifted H" copy keeps every operand
    4B-aligned for the packed mode regardless of the chunk offset parity.
  * The inverse DFT matmuls accumulate DIRECTLY into final output blocks in
    PSUM (overlap-add across frames is re-associated into the matmul
    accumulation), eliminating v1's per-chunk PSUM->SBUF copies and the
    separate shifted-add stage.  The Hann window stays folded into the 32
    inverse matrices; 1/norm is applied by the PSUM-draining DVE multiply.
  * Elementwise work is split DVE/Pool/Act by measured engine rates.

  Data parallel over the batch: 16 rows -> 8 cores x 2 rows.
"""

import numpy as np
import ml_dtypes

import concourse.bass as bass
import concourse.tile as tile
from concourse import bacc
from concourse import mybir
from concourse.bass_utils import run_bass_kernel_spmd
from concourse.masks import make_identity

# problem constants (hardcoded per contract)
HOP, WIN, PAD = 256, 1024, 384
B, T, P = 16, 262144, 22
F = T // HOP              # 1024 frames per row
NFFT = 256
TB = T // 128             # 2048 raw 128-blocks per row
NCORES = 8
BPC = B // NCORES         # 2 batch rows per core
FC = BPC * F              # 2048 frames per core
FTS = 512                 # frames per tile (one PSUM bank at fp32)
XTW = 2056                # xt width: TB + 3 left margin + 5 right
MG = 4                    # X/H/Y tile margins (cols per side)
HW_ = F + 2 * MG          # 1032: per-row width of X/H/Y tiles
J0, J1 = 2, 2 * HW_ - 2   # spectral op col range (both-rows fused)

_f32 = mybir.dt.float32
_f32r = mybir.dt.float32r
_bf16 = mybir.dt.bfloat16

# chunk geometry: chunk c of frame f is raw block b = 2f + c - 3
_PAR = [(c + 1) % 2 for c in range(8)]            # block parity per chunk
_S = [(c - 3 - _PAR[c]) // 2 for c in range(8)]   # X col shift per chunk
_SHIFTED = [s % 2 != 0 for s in _S]               # odd shift -> shifted Y/H


def _r(ap):
    return ap.bitcast(_f32r) if ap.dtype == _f32 else ap


# ---------------------------------------------------------------- constants
def _build_consts():
    n_ = np.arange(128)
    k_ = np.arange(128)
    win = 0.5 * (1.0 - np.cos(2.0 * np.pi * np.arange(WIN) / WIN))

    ang = 2 * np.pi * np.outer(n_, k_) / NFFT
    Fr = np.cos(ang)
    Fi = -np.sin(ang)
    Fi[:, 0] = (-1.0) ** n_                      # slot0: X[128] into Xi[0]

    m_ = np.arange(1, P + 1)
    angA = 2 * np.pi * np.outer(m_, k_) / NFFT
    Ar = np.vstack([np.ones(128), np.cos(angA)])     # [23, 128]
    Ai = np.vstack([np.zeros(128), -np.sin(angA)])
    Ai[:, 0] = (-1.0) ** np.arange(0, P + 1)         # col0: A[128]

    nn = np.arange(256)
    angI = 2 * np.pi * np.outer(k_, nn) / NFFT
    Cr = 2 * np.cos(angI) / NFFT
    Ci = -2 * np.sin(angI) / NFFT
    Cr[0, :] = 1.0 / NFFT
    Ci[0, :] = ((-1.0) ** nn) / NFFT
    INV = np.zeros((128, 8, 4, 128), np.float64)  # [k, c, v, n]
    for blk in range(8):
        wseg = win[128 * blk: 128 * (blk + 1)]
        INV[:, blk, 0, :] = Cr[:, :128] * wseg       # lo-r (chunk = c)
        INV[:, blk, 1, :] = Ci[:, :128] * wseg       # lo-i
        INV[:, blk, 2, :] = Cr[:, 128:] * wseg       # hi-r (chunk = c-1)
        INV[:, blk, 3, :] = Ci[:, 128:] * wseg       # hi-i

    # norm reciprocal, parity split: nrp[p][n, u] = 1/norm at block 2u+p
    idx = (np.arange(F)[:, None] * HOP + np.arange(WIN)[None, :]).reshape(-1)
    L = (F - 1) * HOP + WIN
    norm = np.zeros(L)
    np.add.at(norm, idx, np.tile(win, F))
    nr = (1.0 / norm[PAD:PAD + T]).reshape(TB, 128).T   # [n, block]

    bf = ml_dtypes.bfloat16
    f32 = np.float32
    return {
        "frb": np.ascontiguousarray(Fr, bf),
        "fib": np.ascontiguousarray(Fi, bf),
        "arw": np.ascontiguousarray(Ar, f32),
        "aiw": np.ascontiguousarray(Ai, f32),
        "invw": np.ascontiguousarray(INV.reshape(128, 32 * 128), bf),
        "nre": np.ascontiguousarray(nr[:, 0::2], bf),
        "nro": np.ascontiguousarray(nr[:, 1::2], bf),
    }


# ---------------------------------------------------------------- program
def _emit(nc):
    ex_d = nc.dram_tensor("ex2", [BPC, T], _f32, kind="ExternalInput")
    at_d = nc.dram_tensor("atc", [P + 1, FC], _f32, kind="ExternalInput")
    fr_d = nc.dram_tensor("frb", [128, 128], _bf16, kind="ExternalInput")
    fi_d = nc.dram_tensor("fib", [128, 128], _bf16, kind="ExternalInput")
    ar_d = nc.dram_tensor("arw", [P + 1, 128], _f32, kind="ExternalInput")
    ai_d = nc.dram_tensor("aiw", [P + 1, 128], _f32, kind="ExternalInput")
    inv_d = nc.dram_tensor("invw", [128, 32 * 128], _bf16, kind="ExternalInput")
    nre_d = nc.dram_tensor("nre", [128, TB // 2], _bf16, kind="ExternalInput")
    nro_d = nc.dram_tensor("nro", [128, TB // 2], _bf16, kind="ExternalInput")
    h0_d = nc.dram_tensor("h0", [1, 2 * BPC * HW_], _f32, kind="ExternalInput")
    out_d = nc.dram_tensor("out", [BPC, T], _f32, kind="ExternalOutput")

    with tile.TileContext(nc) as tc:
        _body(nc, tc, ex_d, at_d, fr_d, fi_d, ar_d, ai_d, inv_d,
              nre_d, nro_d, h0_d, out_d)
    return nc


def _drain_row(nc, r, ob, accs, nre, nro, ost, ps, identf, out_d):
    """1/norm multiply (PSUM drain) + de-transpose + 8-way split store."""
    obv = ob[r].rearrange("p (a two) -> p a two", two=2)
    for par in range(2):
        nrt = nre if par == 0 else nro
        for ft in range(2):
            nc.vector.tensor_mul(
                obv[:, bass.ts(ft, FTS), par],
                accs[(r, par, ft)], nrt[:, bass.ts(ft, FTS)])
    for g in range(4):
        st = ost.tile([128, 4, 128], _f32, tag="st")
        pt = ps("ptr")
        for mq in range(4):
            m = 4 * g + mq
            nc.tensor.transpose(
                pt[:, bass.ts(mq, 128)], ob[r][:, bass.ts(m, 128)], identf)
        nc.scalar.copy(st.rearrange("p a b -> p (a b)"), pt)
        for h in range(2):
            nc.sync.dma_start(
                out_d.ap()[r, bass.ds(65536 * g + 32768 * h, 32768)].rearrange(
                    "(m tl n) -> tl m n", m=2, tl=128),
                st[:, bass.ts(h, 2)],
            )


def _body(nc, tc, ex_d, at_d, fr_d, fi_d, ar_d, ai_d, inv_d,
          nre_d, nro_d, h0_d, out_d):
    from contextlib import ExitStack

    with ExitStack() as ctx:
        consts = ctx.enter_context(tc.tile_pool(name="consts", bufs=1))
        big = ctx.enter_context(tc.tile_pool(name="big", bufs=1))
        atp = ctx.enter_context(tc.tile_pool(name="atp", bufs=2))
        xtp = ctx.enter_context(tc.tile_pool(name="xtp", bufs=2))
        raw = ctx.enter_context(tc.tile_pool(name="raw", bufs=8))
        hwk = ctx.enter_context(tc.tile_pool(name="hwk", bufs=1))
        twk = ctx.enter_context(tc.tile_pool(name="twk", bufs=2))
        obp = ctx.enter_context(tc.tile_pool(name="obp", bufs=2))
        ost = ctx.enter_context(tc.tile_pool(name="ost", bufs=2))
        psp = ctx.enter_context(tc.tile_pool(name="psp", bufs=8, space="PSUM"))

        def ps(name):
            return psp.tile([128, FTS], _f32, tag="ps", name=name)

        # ---- constants into SBUF ----
        frb = consts.tile([128, 128], _bf16, tag="frb")
        fib = consts.tile([128, 128], _bf16, tag="fib")
        arw = consts.tile([P + 1, 128], _f32r, tag="arw")
        aiw = consts.tile([P + 1, 128], _f32r, tag="aiw")
        invw = consts.tile([128, 32, 128], _bf16, tag="invw")
        nre = consts.tile([128, TB // 2], _bf16, tag="nre")
        nro = consts.tile([128, TB // 2], _bf16, tag="nro")
        h0t = consts.tile([1, 2, BPC, HW_], _f32, tag="h0t")
        identf = consts.tile([128, 128], _f32, tag="identf")
        nc.sync.dma_start(frb, fr_d.ap())
        nc.sync.dma_start(fib, fi_d.ap())
        nc.sync.dma_start(arw, ar_d.ap().bitcast(_f32r))
        nc.sync.dma_start(aiw, ai_d.ap().bitcast(_f32r))
        for iw in range(4):
            nc.sync.dma_start(
                invw[:, bass.ts(iw, 8)],
                inv_d.ap()[:, bass.ts(iw, 8 * 128)].rearrange(
                    "k (i n) -> k i n", n=128))
        nc.sync.dma_start(nre, nre_d.ap())
        nc.sync.dma_start(nro, nro_d.ap())
        nc.sync.dma_start(h0t, h0_d.ap().rearrange("q (s a b) -> q s a b", s=2, b=HW_))
        make_identity(nc, identf)

        # ---- persistent per-core tensors ----
        xre = big.tile([128, BPC, HW_], _bf16, tag="xre")
        xro = big.tile([128, BPC, HW_], _bf16, tag="xro")
        xie = big.tile([128, BPC, HW_], _bf16, tag="xie")
        xio = big.tile([128, BPC, HW_], _bf16, tag="xio")
        hra = big.tile([128, BPC, HW_], _bf16, tag="hra")
        hrb = big.tile([128, BPC, HW_], _bf16, tag="hrb")
        his = big.tile([128, BPC, HW_], _bf16, tag="his")
        hras = big.tile([128, BPC, HW_], _bf16, tag="hras")
        hrbs = big.tile([128, BPC, HW_], _bf16, tag="hrbs")
        hiss = big.tile([128, BPC, HW_], _bf16, tag="hiss")
        yr = [big.tile([128, BPC, HW_], _bf16, tag=f"yr{c}", name=f"yr{c}")
              for c in range(8)]
        yi = [big.tile([128, BPC, HW_], _bf16, tag=f"yi{c}", name=f"yi{c}")
              for c in range(8)]
        ob = [obp.tile([128, TB], _f32, tag="ob", name="ob0"),
              obp.tile([128, TB], _f32, tag="ob", name="ob1")]


        # ---- input DMAs for both rows, issued up-front ----
        rts = [[None] * 4 for _ in range(BPC)]
        for r in range(BPC):
            for s4 in range(4):
                rt = raw.tile([128, 4, 128], _f32, tag="rt")
                rts[r][s4] = rt
                nc.scalar.dma_start(
                    rt,
                    ex_d.ap()[r, bass.ts(s4, 65536)].rearrange(
                        "(u p j) -> p u j", u=4, p=128),
                )

        # ---- H stage: per-frame filter spectrum H = g / A(w^k) ----
        for ft in range(FC // FTS):
            rsel, fo = ft // (F // FTS), (ft % (F // FTS)) * FTS
            atc = atp.tile([P + 1, FTS], _f32r, tag="atc")
            nc.sync.dma_start(atc, at_d.ap()[:, bass.ts(ft, FTS)].bitcast(_f32r))
            pbr = ps("pbr")
            pbi = ps("pbi")
            nc.tensor.matmul(pbr, arw, atc, start=True, stop=True)
            nc.tensor.matmul(pbi, aiw, atc, start=True, stop=True)
            brs = hwk.tile([128, FTS], _f32, tag="brs")
            bis = hwk.tile([128, FTS], _f32, tag="bis")
            nc.scalar.copy(brs, pbr)
            nc.scalar.copy(bis, pbi)
            t3 = hwk.tile([128, FTS], _f32, tag="t3")
            t4 = hwk.tile([128, FTS], _f32, tag="t4")
            nc.vector.tensor_mul(t3, brs, brs)
            nc.vector.tensor_mul(t4, bis, bis)
            nc.vector.tensor_add(t3, t3, t4)
            nc.vector.reciprocal_approx_fast(t4, t3)
            ds = bass.ds(MG + fo, FTS)
            nc.vector.tensor_mul(hra[:, rsel, ds], brs, t4)
            nc.vector.tensor_mul(his[:, rsel, ds], bis, t4)
        # margins: zero once; interior writes never touch them
        for tl in [xre, xro, xie, xio, hra, hrb, his]:
            fl = tl.rearrange("p a b -> p (a b)")
            nc.gpsimd.memset(fl[:, 0:MG], 0.0)
            nc.gpsimd.memset(fl[:, HW_ - MG:HW_ + MG], 0.0)
            nc.gpsimd.memset(fl[:, 2 * HW_ - MG:2 * HW_], 0.0)
        for tl in yr + yi:
            fl = tl.rearrange("p a b -> p (a b)")
            # cols {0,1, 1030,1031, 1032,1033, 2062,2063}
            v = fl.rearrange("p (a b) -> p a b", b=2)
            nc.gpsimd.memset(v[:, 0:1, :], 0.0)
            nc.gpsimd.memset(v[:, 515:517, :], 0.0)
            nc.gpsimd.memset(v[:, 1031:1032, :], 0.0)
        # slot-0 rows: his[0]=0; hra[0]=H(1), hrb[0]=H(-1) from host; the
        # hrb body rows are a copy of hra (both are Re(H) off slot 0)
        nc.gpsimd.memset(his[0:1], 0.0)
        nc.scalar.copy(hra[0:1], h0t[:, 0])

        def _h_shifts():
            # hrb + shifted-H copies, emitted after row-0 X so the Act engine
            # prioritizes the input pipeline
            nc.scalar.copy(hrb, hra)
            nc.scalar.copy(hrb[0:1], h0t[:, 1])
            W2 = 2 * HW_
            for src_, dst in ((hra, hras), (hrb, hrbs), (his, hiss)):
                sf = src_.rearrange("p a b -> p (a b)")
                df = dst.rearrange("p a b -> p (a b)")
                nc.scalar.copy(df[:, 0:W2 - 1], sf[:, 1:W2])
                nc.gpsimd.memset(df[:, W2 - 1:W2], 0.0)

        # ---- transpose + shared forward FFT (per row) ----
        for r in range(BPC):
            xt = xtp.tile([128, XTW], _bf16, tag="xt")
            nc.vector.memset(xt[:, 0:3], 0.0)
            nc.vector.memset(xt[:, 3 + TB:XTW], 0.0)
            for s4 in range(4):
                pt = ps("pt")
                for q in range(4):
                    nc.tensor.transpose(
                        pt[:, bass.ts(q, 128)], rts[r][s4][:, q], identf)
                nc.scalar.copy(
                    xt[:, 3 + 512 * s4: 3 + 512 * (s4 + 1)], pt)
            xtv = xt.rearrange("p (t two) -> p two t", two=2)
            # parity-direct forward FFT: q=1 cols are even blocks b=2t'-2,
            # q=0 cols are odd blocks b=2t'-3
            tail = ps("tail")
            ti = 0
            for q, coff, xr_, xi_ in ((1, MG - 1, xre, xie),
                                      (0, MG - 2, xro, xio)):
                for w, xdst in ((frb, xr_), (fib, xi_)):
                    for k in range(2):
                        px = ps("px")
                        nc.tensor.matmul(px, w, xtv[:, q, bass.ts(k, 512)],
                                         start=True, stop=True)
                        nc.scalar.copy(
                            xdst[:, r, bass.ds(coff + 512 * k, 512)], px)
                    # 4-col tail piece into shared tail psum
                    nc.tensor.matmul(tail[:, bass.ds(4 * ti, 4)], w,
                                     xtv[:, q, bass.ds(1024, 4)],
                                     start=True, stop=True)
                    nc.scalar.copy(xdst[:, r, bass.ds(coff + 1024, 4)],
                                   tail[:, bass.ds(4 * ti, 4)])
                    ti += 1
            if r == 0:
                _h_shifts()

        # ---- spectral multiply + inverse DFT/overlap-add, chunk-pipelined --
        accs = {}
        nstop = {}
        for r in range(BPC):
            for par in range(2):
                for ft in range(2):
                    cs = [c for c in range(8) if c % 2 == (par + 3) % 2]
                    n = sum(2 if (c == 0) else 4 for c in cs)
                    nstop[(r, par, ft)] = n

        WSP = HW_ - MG  # 1028: spectral op width per row
        nterm = {}
        for r in range(BPC):
            for c in range(8):
                if r == 1 and c == 1:
                    # row-0 drain + store overlaps row-1 spectral/inverse
                    _drain_row(nc, 0, ob, accs, nre, nro, ost, ps, identf,
                               out_d)
                par, s, sh = _PAR[c], _S[c], _SHIFTED[c]
                xr_ = xre if par == 0 else xro
                xi_ = xie if par == 0 else xio
                h1 = hras if sh else hra
                h2 = hrbs if sh else hrb
                h3 = hiss if sh else his
                so = s + (1 if sh else 0)
                xsl = bass.ds(J0 + so, WSP)
                hsl = bass.ds(J0, WSP)
                t1 = twk.tile([128, WSP], _bf16, tag="t1")
                t2 = twk.tile([128, WSP], _bf16, tag="t2")
                t3_ = twk.tile([128, WSP], _bf16, tag="t3")
                t4_ = twk.tile([128, WSP], _bf16, tag="t4")
                nc.vector.tensor_mul(t1, xr_[:, r, xsl], h1[:, r, hsl])
                nc.vector.tensor_mul(t2, xi_[:, r, xsl], h3[:, r, hsl])
                nc.vector.tensor_mul(t3_, xi_[:, r, xsl], h2[:, r, hsl])
                nc.vector.tensor_mul(t4_, xr_[:, r, xsl], h3[:, r, hsl])
                nc.vector.tensor_add(yr[c][:, r, hsl], t1, t2)
                nc.vector.tensor_sub(yi[c][:, r, hsl], t3_, t4_)

                # inverse terms that become ready with this chunk:
                # lo terms of chunk c and hi terms of chunk c (reading c-1)
                cpar = (c + 1) % 2   # block parity this chunk contributes to
                for ft in range(2):
                    key = (r, cpar, ft)
                    if key not in accs:
                        accs[key] = ps(f"acc{r}{cpar}{ft}")
                        nterm[key] = 0
                    b0 = cpar + 1024 * ft
                    sc = (b0 + 3 - c) // 2
                    for v, cc in ((0, c), (1, c), (2, c - 1), (3, c - 1)):
                        if cc < 0:
                            continue
                        ysrc = yr[cc] if v % 2 == 0 else yi[cc]
                        col = MG + sc - (1 if _SHIFTED[cc] else 0)
                        nterm[key] += 1
                        nc.tensor.matmul(
                            accs[key], invw[:, 4 * c + v],
                            ysrc[:, r, bass.ds(col, FTS)],
                            start=(nterm[key] == 1),
                            stop=(nterm[key] == nstop[key]),
                        )
        _drain_row(nc, 1, ob, accs, nre, nro, ost, ps, identf, out_d)


# ---------------------------------------------------------------- entry
_prog = None


def _get_program():
    global _prog
    if _prog is None:
        nc = bacc.Bacc("TRN2", target_bir_lowering=False, debug=False)
        _prog = _emit(nc)
        nc.compile()
    return _prog


def build_in_maps(ex, gain, a):
    ex = np.ascontiguousarray(ex, np.float32)
    gain = np.ascontiguousarray(gain, np.float32)
    a = np.ascontiguousarray(a, np.float32)
    consts = _build_consts()

    # host prep of the tiny per-frame coefficient tensor: [1, a]/g -> [23, F]
    at = np.concatenate([np.ones((B, F, 1), np.float32), a], axis=2)
    at /= gain[:, :, None]
    # slot-0 filter rows, exact on host: H(w=1) = 1/sum(at), H(w=-1) alt-sum
    sgn = (-1.0) ** np.arange(P + 1, dtype=np.float32)
    br0 = at.sum(axis=2)
    bi0 = (at * sgn).sum(axis=2)
    h0 = np.zeros((B, 2, HW_), np.float32)
    h0[:, 0, MG:MG + F] = 1.0 / br0
    h0[:, 1, MG:MG + F] = 1.0 / bi0

    in_maps = []
    for c in range(NCORES):
        rows = slice(BPC * c, BPC * (c + 1))
        in_maps.append({
            "ex2": ex[rows],
            "atc": np.ascontiguousarray(
                at[rows].reshape(FC, P + 1).T, np.float32),
            "h0": np.ascontiguousarray(
                h0[rows].transpose(1, 0, 2).reshape(1, 2 * BPC * HW_)),
            **consts,
        })
    return in_maps


def kernel(ex: np.ndarray, gain: np.ndarray, a: np.ndarray) -> np.ndarray:
    nc = _get_program()
    in_maps = build_in_maps(ex, gain, a)
    res = run_bass_kernel_spmd(nc, in_maps, list(range(NCORES)))
    out = np.concatenate([res.results[i]["out"] for i in range(NCORES)], axis=0)
    return np.ascontiguousarray(out, np.float32)


if __name__ == "__main__":
    rng = np.random.default_rng(0)
    y = kernel(
        rng.standard_normal((B, T), dtype=np.float32),
        rng.uniform(0.1, 1.0, (B, F)).astype(np.float32),
        (rng.standard_normal((B, F, P), dtype=np.float32) * 0.01),
    )
    print(y.shape, y.dtype, float(np.abs(y).max()))


# revision 27
# speedup vs baseline: 1.1633x; 1.1311x over previous
"""Trainium2 Bass kernel for batched per-frame LPC synthesis + windowed overlap-add.

v2 — restructured from the v1 FFT kernel for engine balance:
  * The forward FFT of each 128-sample block is computed ONCE per block and
    shared by the 4 overlapping frames that contain it (v1 recomputed it 4x).
  * Spectral multiply Y = X*H runs on DVE in bf16 (2x packed mode) with
    row-fused long instructions; a "shifted H" copy keeps every operand
    4B-aligned for the packed mode regardless of the chunk offset parity.
  * The inverse DFT matmuls accumulate DIRECTLY into final output blocks in
    PSUM (overlap-add across frames is re-associated into the matmul
    accumulation), eliminating v1's per-chunk PSUM->SBUF copies and the
    separate shifted-add stage.  The Hann window stays folded into the 32
    inverse matrices; 1/norm is applied by the PSUM-draining DVE multiply.
  * Elementwise work is split DVE/Pool/Act by measured engine rates.

  Data parallel over the batch: 16 rows -> 8 cores x 2 rows.
"""

import numpy as np
import ml_dtypes

import concourse.bass as bass
import concourse.tile as tile
from concourse import bacc
from concourse import mybir
from concourse.bass_utils import run_bass_kernel_spmd
from concourse.masks import make_identity

# problem constants (hardcoded per contract)
HOP, WIN, PAD = 256, 1024, 384
B, T, P = 16, 262144, 22
F = T // HOP              # 1024 frames per row
NFFT = 256
TB = T // 128             # 2048 raw 128-blocks per row
NCORES = 8
BPC = B // NCORES         # 2 batch rows per core
FC = BPC * F              # 2048 frames per core
FTS = 512                 # frames per tile (one PSUM bank at fp32)
XTW = 2056                # xt width: TB + 3 left margin + 5 right
MG = 4                    # X/H/Y tile margins (cols per side)
HW_ = F + 2 * MG          # 1032: per-row width of X/H/Y tiles
J0, J1 = 2, 2 * HW_ - 2   # spectral op col range (both-rows fused)

_f32 = mybir.dt.float32
_f32r = mybir.dt.float32r
_bf16 = mybir.dt.bfloat16

# chunk geometry: chunk c of frame f is raw block b = 2f + c - 3
_PAR = [(c + 1) % 2 for c in range(8)]            # block parity per chunk
_S = [(c - 3 - _PAR[c]) // 2 for c in range(8)]   # X col shift per chunk
_SHIFTED = [s % 2 != 0 for s in _S]               # odd shift -> shifted Y/H


def _r(ap):
    return ap.bitcast(_f32r) if ap.dtype == _f32 else ap


# ---------------------------------------------------------------- constants
def _build_consts():
    n_ = np.arange(128)
    k_ = np.arange(128)
    win = 0.5 * (1.0 - np.cos(2.0 * np.pi * np.arange(WIN) / WIN))

    ang = 2 * np.pi * np.outer(n_, k_) / NFFT
    Fr = np.cos(ang)
    Fi = -np.sin(ang)
    Fi[:, 0] = (-1.0) ** n_                      # slot0: X[128] into Xi[0]

    m_ = np.arange(1, P + 1)
    angA = 2 * np.pi * np.outer(m_, k_) / NFFT
    Ar = np.vstack([np.ones(128), np.cos(angA)])     # [23, 128]
    Ai = np.vstack([np.zeros(128), -np.sin(angA)])
    Ai[:, 0] = (-1.0) ** np.arange(0, P + 1)         # col0: A[128]

    nn = np.arange(256)
    angI = 2 * np.pi * np.outer(k_, nn) / NFFT
    Cr = 2 * np.cos(angI) / NFFT
    Ci = -2 * np.sin(angI) / NFFT
    Cr[0, :] = 1.0 / NFFT
    Ci[0, :] = ((-1.0) ** nn) / NFFT
    INV = np.zeros((128, 8, 4, 128), np.float64)  # [k, c, v, n]
    for blk in range(8):
        wseg = win[128 * blk: 128 * (blk + 1)]
        INV[:, blk, 0, :] = Cr[:, :128] * wseg       # lo-r (chunk = c)
        INV[:, blk, 1, :] = Ci[:, :128] * wseg       # lo-i
        INV[:, blk, 2, :] = Cr[:, 128:] * wseg       # hi-r (chunk = c-1)
        INV[:, blk, 3, :] = Ci[:, 128:] * wseg       # hi-i

    # norm reciprocal, parity split: nrp[p][n, u] = 1/norm at block 2u+p
    idx = (np.arange(F)[:, None] * HOP + np.arange(WIN)[None, :]).reshape(-1)
    L = (F - 1) * HOP + WIN
    norm = np.zeros(L)
    np.add.at(norm, idx, np.tile(win, F))
    nr = (1.0 / norm[PAD:PAD + T]).reshape(TB, 128).T   # [n, block]

    bf = ml_dtypes.bfloat16
    f32 = np.float32
    return {
        "frb": np.ascontiguousarray(Fr, bf),
        "fib": np.ascontiguousarray(Fi, bf),
        "arw": np.ascontiguousarray(Ar, f32),
        "aiw": np.ascontiguousarray(Ai, f32),
        "invw": np.ascontiguousarray(INV.reshape(128, 32 * 128), bf),
        "nre": np.ascontiguousarray(nr[:, 0::2], bf),
        "nro": np.ascontiguousarray(nr[:, 1::2], bf),
    }


# ---------------------------------------------------------------- program
def _emit(nc):
    ex_d = nc.dram_tensor("ex2", [BPC, T], _f32, kind="ExternalInput")
    at_d = nc.dram_tensor("atc", [P + 1, FC], _f32, kind="ExternalInput")
    fr_d = nc.dram_tensor("frb", [128, 128], _bf16, kind="ExternalInput")
    fi_d = nc.dram_tensor("fib", [128, 128], _bf16, kind="ExternalInput")
    ar_d = nc.dram_tensor("arw", [P + 1, 128], _f32, kind="ExternalInput")
    ai_d = nc.dram_tensor("aiw", [P + 1, 128], _f32, kind="ExternalInput")
    inv_d = nc.dram_tensor("invw", [128, 32 * 128], _bf16, kind="ExternalInput")
    nre_d = nc.dram_tensor("nre", [128, TB // 2], _bf16, kind="ExternalInput")
    nro_d = nc.dram_tensor("nro", [128, TB // 2], _bf16, kind="ExternalInput")
    h0_d = nc.dram_tensor("h0", [1, 2 * BPC * HW_], _f32, kind="ExternalInput")
    out_d = nc.dram_tensor("out", [BPC, T], _f32, kind="ExternalOutput")

    with tile.TileContext(nc) as tc:
        _body(nc, tc, ex_d, at_d, fr_d, fi_d, ar_d, ai_d, inv_d,
              nre_d, nro_d, h0_d, out_d)
    return nc


def _drain_row(nc, r, ob, accs, nre, nro, ost, ps, identf, out_d):
    """1/norm multiply (PSUM drain) + de-transpose + 8-way split store."""
    obv = ob[r].rearrange("p (a two) -> p a two", two=2)
    for par in range(2):
        nrt = nre if par == 0 else nro
        for ft in range(2):
            nc.vector.tensor_mul(
                obv[:, bass.ts(ft, FTS), par],
                accs[(r, par, ft)], nrt[:, bass.ts(ft, FTS)])
    for g in range(4):
        st = ost.tile([128, 4, 128], _f32, tag="st")
        pt = ps("ptr")
        for mq in range(4):
            m = 4 * g + mq
            nc.tensor.transpose(
                pt[:, bass.ts(mq, 128)], ob[r][:, bass.ts(m, 128)], identf)
        nc.scalar.copy(st.rearrange("p a b -> p (a b)"), pt)
        for h in range(2):
            nc.sync.dma_start(
                out_d.ap()[r, bass.ds(65536 * g + 32768 * h, 32768)].rearrange(
                    "(m tl n) -> tl m n", m=2, tl=128),
                st[:, bass.ts(h, 2)],
            )


def _body(nc, tc, ex_d, at_d, fr_d, fi_d, ar_d, ai_d, inv_d,
          nre_d, nro_d, h0_d, out_d):
    from contextlib import ExitStack

    with ExitStack() as ctx:
        consts = ctx.enter_context(tc.tile_pool(name="consts", bufs=1))
        big = ctx.enter_context(tc.tile_pool(name="big", bufs=1))
        atp = ctx.enter_context(tc.tile_pool(name="atp", bufs=2))
        xtp = ctx.enter_context(tc.tile_pool(name="xtp", bufs=2))
        raw = ctx.enter_context(tc.tile_pool(name="raw", bufs=8))
        hwk = ctx.enter_context(tc.tile_pool(name="hwk", bufs=1))
        twk = ctx.enter_context(tc.tile_pool(name="twk", bufs=2))
        obp = ctx.enter_context(tc.tile_pool(name="obp", bufs=2))
        ost = ctx.enter_context(tc.tile_pool(name="ost", bufs=2))
        psp = ctx.enter_context(tc.tile_pool(name="psp", bufs=8, space="PSUM"))

        def ps(name):
            return psp.tile([128, FTS], _f32, tag="ps", name=name)

        # ---- constants into SBUF ----
        frb = consts.tile([128, 128], _bf16, tag="frb")
        fib = consts.tile([128, 128], _bf16, tag="fib")
        arw = consts.tile([P + 1, 128], _f32r, tag="arw")
        aiw = consts.tile([P + 1, 128], _f32r, tag="aiw")
        invw = consts.tile([128, 32, 128], _bf16, tag="invw")
        nre = consts.tile([128, TB // 2], _bf16, tag="nre")
        nro = consts.tile([128, TB // 2], _bf16, tag="nro")
        h0t = consts.tile([1, 2, BPC, HW_], _f32, tag="h0t")
        identf = consts.tile([128, 128], _f32, tag="identf")
        nc.sync.dma_start(frb, fr_d.ap())
        nc.sync.dma_start(fib, fi_d.ap())
        nc.sync.dma_start(arw, ar_d.ap().bitcast(_f32r))
        nc.sync.dma_start(aiw, ai_d.ap().bitcast(_f32r))
        for iw in range(4):
            nc.sync.dma_start(
                invw[:, bass.ts(iw, 8)],
                inv_d.ap()[:, bass.ts(iw, 8 * 128)].rearrange(
                    "k (i n) -> k i n", n=128))
        nc.sync.dma_start(nre, nre_d.ap())
        nc.sync.dma_start(nro, nro_d.ap())
        nc.sync.dma_start(h0t, h0_d.ap().rearrange("q (s a b) -> q s a b", s=2, b=HW_))
        make_identity(nc, identf)

        # ---- persistent per-core tensors ----
        xre = big.tile([128, BPC, HW_], _bf16, tag="xre")
        xro = big.tile([128, BPC, HW_], _bf16, tag="xro")
        xie = big.tile([128, BPC, HW_], _bf16, tag="xie")
        xio = big.tile([128, BPC, HW_], _bf16, tag="xio")
        hra = big.tile([128, BPC, HW_], _bf16, tag="hra")
        hrb = big.tile([128, BPC, HW_], _bf16, tag="hrb")
        his = big.tile([128, BPC, HW_], _bf16, tag="his")
        hras = big.tile([128, BPC, HW_], _bf16, tag="hras")
        hrbs = big.tile([128, BPC, HW_], _bf16, tag="hrbs")
        hiss = big.tile([128, BPC, HW_], _bf16, tag="hiss")
        yr = [big.tile([128, BPC, HW_], _bf16, tag=f"yr{c}", name=f"yr{c}")
              for c in range(8)]
        yi = [big.tile([128, BPC, HW_], _bf16, tag=f"yi{c}", name=f"yi{c}")
              for c in range(8)]
        ob = [obp.tile([128, TB], _f32, tag="ob", name="ob0"),
              obp.tile([128, TB], _f32, tag="ob", name="ob1")]


        # ---- input DMAs for both rows, issued up-front ----
        rts = [[None] * 4 for _ in range(BPC)]
        for r in range(BPC):
            for s4 in range(4):
                rt = raw.tile([128, 4, 128], _f32, tag="rt")
                rts[r][s4] = rt
                nc.scalar.dma_start(
                    rt,
                    ex_d.ap()[r, bass.ts(s4, 65536)].rearrange(
                        "(u p j) -> p u j", u=4, p=128),
                )

        # ---- H stage: per-frame filter spectrum H = g / A(w^k) ----
        for ft in range(FC // FTS):
            rsel, fo = ft // (F // FTS), (ft % (F // FTS)) * FTS
            atc = atp.tile([P + 1, FTS], _f32r, tag="atc")
            nc.sync.dma_start(atc, at_d.ap()[:, bass.ts(ft, FTS)].bitcast(_f32r))
            pbr = ps("pbr")
            pbi = ps("pbi")
            nc.tensor.matmul(pbr, arw, atc, start=True, stop=True)
            nc.tensor.matmul(pbi, aiw, atc, start=True, stop=True)
            brs = hwk.tile([128, FTS], _f32, tag="brs")
            bis = hwk.tile([128, FTS], _f32, tag="bis")
            nc.scalar.copy(brs, pbr)
            nc.scalar.copy(bis, pbi)
            t3 = hwk.tile([128, FTS], _f32, tag="t3")
            t4 = hwk.tile([128, FTS], _f32, tag="t4")
            nc.gpsimd.tensor_mul(t3, brs, brs)
            nc.gpsimd.tensor_mul(t4, bis, bis)
            nc.gpsimd.tensor_add(t3, t3, t4)
            nc.vector.reciprocal_approx_fast(t4, t3)
            ds = bass.ds(MG + fo, FTS)
            nc.vector.tensor_mul(hra[:, rsel, ds], brs, t4)
            nc.vector.tensor_mul(his[:, rsel, ds], bis, t4)
        # margins: zero once; interior writes never touch them
        for tl in [xre, xro, xie, xio, hra, hrb, his]:
            fl = tl.rearrange("p a b -> p (a b)")
            nc.gpsimd.memset(fl[:, 0:MG], 0.0)
            nc.gpsimd.memset(fl[:, HW_ - MG:HW_ + MG], 0.0)
            nc.gpsimd.memset(fl[:, 2 * HW_ - MG:2 * HW_], 0.0)
        for tl in yr + yi:
            fl = tl.rearrange("p a b -> p (a b)")
            # cols {0,1, 1030,1031, 1032,1033, 2062,2063}
            v = fl.rearrange("p (a b) -> p a b", b=2)
            nc.gpsimd.memset(v[:, 0:1, :], 0.0)
            nc.gpsimd.memset(v[:, 515:517, :], 0.0)
            nc.gpsimd.memset(v[:, 1031:1032, :], 0.0)
        # slot-0 rows: his[0]=0; hra[0]=H(1), hrb[0]=H(-1) from host; the
        # hrb body rows are a copy of hra (both are Re(H) off slot 0)
        nc.gpsimd.memset(his[0:1], 0.0)
        nc.scalar.copy(hra[0:1], h0t[:, 0])

        def _h_shifts():
            # hrb + shifted-H copies, emitted after row-0 X so the Act engine
            # prioritizes the input pipeline
            nc.scalar.copy(hrb, hra)
            nc.scalar.copy(hrb[0:1], h0t[:, 1])
            W2 = 2 * HW_
            for src_, dst in ((hra, hras), (hrb, hrbs), (his, hiss)):
                sf = src_.rearrange("p a b -> p (a b)")
                df = dst.rearrange("p a b -> p (a b)")
                nc.scalar.copy(df[:, 0:W2 - 1], sf[:, 1:W2])
                nc.gpsimd.memset(df[:, W2 - 1:W2], 0.0)

        # ---- transpose + shared forward FFT (per row) ----
        for r in range(BPC):
            xt = xtp.tile([128, XTW], _bf16, tag="xt")
            nc.vector.memset(xt[:, 0:3], 0.0)
            nc.vector.memset(xt[:, 3 + TB:XTW], 0.0)
            for s4 in range(4):
                pt = ps("pt")
                for q in range(4):
                    nc.tensor.transpose(
                        pt[:, bass.ts(q, 128)], rts[r][s4][:, q], identf)
                nc.scalar.copy(
                    xt[:, 3 + 512 * s4: 3 + 512 * (s4 + 1)], pt)
            xtv = xt.rearrange("p (t two) -> p two t", two=2)
            # parity-direct forward FFT: q=1 cols are even blocks b=2t'-2,
            # q=0 cols are odd blocks b=2t'-3
            tail = ps("tail")
            ti = 0
            for q, coff, xr_, xi_ in ((1, MG - 1, xre, xie),
                                      (0, MG - 2, xro, xio)):
                for w, xdst in ((frb, xr_), (fib, xi_)):
                    for k in range(2):
                        px = ps("px")
                        nc.tensor.matmul(px, w, xtv[:, q, bass.ts(k, 512)],
                                         start=True, stop=True)
                        nc.scalar.copy(
                            xdst[:, r, bass.ds(coff + 512 * k, 512)], px)
                    # 4-col tail piece into shared tail psum
                    nc.tensor.matmul(tail[:, bass.ds(4 * ti, 4)], w,
                                     xtv[:, q, bass.ds(1024, 4)],
                                     start=True, stop=True)
                    nc.scalar.copy(xdst[:, r, bass.ds(coff + 1024, 4)],
                                   tail[:, bass.ds(4 * ti, 4)])
                    ti += 1
            if r == 0:
                _h_shifts()

        # ---- spectral multiply + inverse DFT/overlap-add, chunk-pipelined --
        accs = {}
        nstop = {}
        for r in range(BPC):
            for par in range(2):
                for ft in range(2):
                    cs = [c for c in range(8) if c % 2 == (par + 3) % 2]
                    n = sum(2 if (c == 0) else 4 for c in cs)
                    nstop[(r, par, ft)] = n

        WSP = HW_ - MG  # 1028: spectral op width per row
        nterm = {}
        for r in range(BPC):
            for c in range(8):
                if r == 1 and c == 1:
                    # row-0 drain + store overlaps row-1 spectral/inverse
                    _drain_row(nc, 0, ob, accs, nre, nro, ost, ps, identf,
                               out_d)
                par, s, sh = _PAR[c], _S[c], _SHIFTED[c]
                xr_ = xre if par == 0 else xro
                xi_ = xie if par == 0 else xio
                h1 = hras if sh else hra
                h2 = hrbs if sh else hrb
                h3 = hiss if sh else his
                so = s + (1 if sh else 0)
                xsl = bass.ds(J0 + so, WSP)
                hsl = bass.ds(J0, WSP)
                t1 = twk.tile([128, WSP], _bf16, tag="t1")
                t2 = twk.tile([128, WSP], _bf16, tag="t2")
                t3_ = twk.tile([128, WSP], _bf16, tag="t3")
                t4_ = twk.tile([128, WSP], _bf16, tag="t4")
                nc.vector.tensor_mul(t1, xr_[:, r, xsl], h1[:, r, hsl])
                nc.vector.tensor_mul(t2, xi_[:, r, xsl], h3[:, r, hsl])
                nc.vector.tensor_mul(t3_, xi_[:, r, xsl], h2[:, r, hsl])
                nc.vector.tensor_mul(t4_, xr_[:, r, xsl], h3[:, r, hsl])
                nc.vector.tensor_add(yr[c][:, r, hsl], t1, t2)
                nc.vector.tensor_sub(yi[c][:, r, hsl], t3_, t4_)

                # inverse terms that become ready with this chunk:
                # lo terms of chunk c and hi terms of chunk c (reading c-1)
                cpar = (c + 1) % 2   # block parity this chunk contributes to
                for ft in range(2):
                    key = (r, cpar, ft)
                    if key not in accs:
                        accs[key] = ps(f"acc{r}{cpar}{ft}")
                        nterm[key] = 0
                    b0 = cpar + 1024 * ft
                    sc = (b0 + 3 - c) // 2
                    for v, cc in ((0, c), (1, c), (2, c - 1), (3, c - 1)):
                        if cc < 0:
                            continue
                        ysrc = yr[cc] if v % 2 == 0 else yi[cc]
                        col = MG + sc - (1 if _SHIFTED[cc] else 0)
                        nterm[key] += 1
                        nc.tensor.matmul(
                            accs[key], invw[:, 4 * c + v],
                            ysrc[:, r, bass.ds(col, FTS)],
                            start=(nterm[key] == 1),
                            stop=(nterm[key] == nstop[key]),
                        )
        _drain_row(nc, 1, ob, accs, nre, nro, ost, ps, identf, out_d)


# ---------------------------------------------------------------- entry
_prog = None


def _get_program():
    global _prog
    if _prog is None:
        nc = bacc.Bacc("TRN2", target_bir_lowering=False, debug=False)
        _prog = _emit(nc)
        nc.compile()
    return _prog


def build_in_maps(ex, gain, a):
    ex = np.ascontiguousarray(ex, np.float32)
    gain = np.ascontiguousarray(gain, np.float32)
    a = np.ascontiguousarray(a, np.float32)
    consts = _build_consts()

    # host prep of the tiny per-frame coefficient tensor: [1, a]/g -> [23, F]
    at = np.concatenate([np.ones((B, F, 1), np.float32), a], axis=2)
    at /= gain[:, :, None]
    # slot-0 filter rows, exact on host: H(w=1) = 1/sum(at), H(w=-1) alt-sum
    sgn = (-1.0) ** np.arange(P + 1, dtype=np.float32)
    br0 = at.sum(axis=2)
    bi0 = (at * sgn).sum(axis=2)
    h0 = np.zeros((B, 2, HW_), np.float32)
    h0[:, 0, MG:MG + F] = 1.0 / br0
    h0[:, 1, MG:MG + F] = 1.0 / bi0

    in_maps = []
    for c in range(NCORES):
        rows = slice(BPC * c, BPC * (c + 1))
        in_maps.append({
            "ex2": ex[rows],
            "atc": np.ascontiguousarray(
                at[rows].reshape(FC, P + 1).T, np.float32),
            "h0": np.ascontiguousarray(
                h0[rows].transpose(1, 0, 2).reshape(1, 2 * BPC * HW_)),
            **consts,
        })
    return in_maps


def kernel(ex: np.ndarray, gain: np.ndarray, a: np.ndarray) -> np.ndarray:
    nc = _get_program()
    in_maps = build_in_maps(ex, gain, a)
    res = run_bass_kernel_spmd(nc, in_maps, list(range(NCORES)))
    out = np.concatenate([res.results[i]["out"] for i in range(NCORES)], axis=0)
    return np.ascontiguousarray(out, np.float32)


if __name__ == "__main__":
    rng = np.random.default_rng(0)
    y = kernel(
        rng.standard_normal((B, T), dtype=np.float32),
        rng.uniform(0.1, 1.0, (B, F)).astype(np.float32),
        (rng.standard_normal((B, F, P), dtype=np.float32) * 0.01),
    )
    print(y.shape, y.dtype, float(np.abs(y).max()))
